# revision 66
# baseline (speedup 1.0000x reference)
"""Trainium2 Bass kernel for ExpertMLPLoRA (moe_routing).

Reference computation (per batch b, selected expert k):
    A = A_all[expert_indices]            # [K, D, R]
    Bm = B_all[expert_indices]           # [K, R, D]
    down = einsum('bkmd,kdr->bkmr', z, A)
    up   = einsum('bkmr,krd->bkmd', down, Bm)
    out  = up * (alpha/rank)

Sharding: data-parallel over batch B=8 -> one batch per NeuronCore.
Each core receives its z[b] slice plus the full (replicated) LoRA
tables and gathers the K=8 selected experts on-device via indirect
DMA.  Host only pre-expands the 8 expert indices into row indices
(pure address arithmetic).

Device pipeline per (b, k):
  1. SWDGE cast-DMA z[b,k] [512, 1024] f32 HBM -> bf16 SBUF with one
     fully-contiguous 16 KiB read run per partition (partition p holds
     rows 4p..4p+3, i.e. m = 4*mp + ms) -- minimal descriptor count
  2. 32x PE transpose (bf16) -> z^T chunks in PSUM, copied to SBUF on
     DVE/ACT; the m axis stays in the permuted (ms, mp) order
  3. mm1: one PSUM accumulation group of 8 matmuls
     A_chunk[128d,16r].T @ z^T[128d,512m] += down^T.  A throwaway PE
     transpose reading one column of every z^T chunk gates the group:
     every member is ready the moment the group opens, so the tile
     scheduler cannot interleave a foreign matmul into the open group
     (that faults this hardware; _assert_contiguous_acc_groups checks
     the final schedule statically).
  4. one PSUM->SBUF bf16 copy of down^T (LoRA scale folded into the
     gathered A table)
  5. mm2 per m-chunk ms: up[128mp, 1024d] = down^T[16,128].T @
     B_k[16,1024]; PSUM->SBUF copy, then a 512 KiB store with the
     permuted row mapping out[k, 4*mp+ms, :]
"""

import numpy as np

_B, _K, _M, _D, _R = 8, 8, 512, 1024, 16
_SCALE = 1.0 / _R
_NCORES = 8

_cache = {}


def _apply_tile_drain_patch():
    """This walrus build caps sync waits at 1 per instruction (2 for
    EventSemaphore).  Tile's kernel-tail drain piles every final sem wait
    onto one Drain -> NCC_INLA001 'Too many sync wait commands'.  Re-emit
    the extras as standalone per-sem waits before the drain."""
    import concourse.tile as tile_mod
    from concourse.tile import TileContext

    if getattr(TileContext, "_drain_patch_applied", False):
        return
    try:
        from concourse.tile import ScopedClock
    except ImportError:
        from bass_rust import ScopedClock

    def _patched(self, tick_clock, wait_clock):
        nc = self.nc
        probe = nc.sync.drain()
        wait_clock.add_sem_waits(
            probe.ins, ScopedClock({None: tick_clock.global_clock})
        )
        waits = list(probe.ins.sync_info.on_wait)
        if len(waits) > 1:
            assert self.sems is not None
            by_name = {s.name: s for s in self.sems.allocated().values()}
            for w in waits[1:]:
                sem = by_name.get(w.ant_name)
                assert sem is not None, f"semaphore {w.ant_name} not found"
                nc.sync.wait_ge(sem, w.wait_value)
            probe.ins.sync_info.on_wait = waits[:1]
            nc.sync.drain()
        nc.all_engine_barrier()
        assert self.sems is not None
        popped = nc._tile_sem_poison_stack.pop()
        assert popped is self._sem_poison
        # End-of-kernel sem/DMA-queue scrubbing (gpsimd dma_reset +
        # sem_clear + a second all-engine barrier) is only needed if the
        # NEFF re-executes; this kernel is single-shot, so do only the
        # Python-side bookkeeping of clear_and_free_semaphores and skip
        # the ~3-5us of teardown instructions.
        sems = list(self.sems.allocated().values())
        sem_nums = [s.num for s in sems]
        if sem_nums:
            nc._state.prepend_free_semaphores(sem_nums)
            for poison_set in nc._tile_sem_poison_stack:
                poison_set.update(sem_nums)

    TileContext._drain_and_barrier = _patched
    TileContext._drain_patch_applied = True


def _split_excess_waits(nc):
    """This walrus build rejects instructions carrying more than 1-2 sync
    waits ('Too many sync wait commands'), but Tile's sem-assignment packs
    up to ~9 waits onto one instruction.  Hoist the excess onto standalone
    EventSemaphore carriers placed immediately before the instruction on
    the same engine (engines execute in order, so blocking semantics are
    identical)."""
    import bass_rust
    import concourse.mybir as mybir

    n = 0
    for fn in nc.m.functions:
        for bb in fn.blocks:
            new_insts = []
            for inst in bb.instructions:
                si = inst.sync_info
                waits = list(si.on_wait) if si is not None else []
                cap = 2 if isinstance(inst, mybir.InstEventSemaphore) else 1
                if len(waits) > cap:
                    for w in waits[cap:]:
                        n += 1
                        new_insts.append(
                            mybir.InstEventSemaphore(
                                name=f"wsplit-{n}-{inst.name}",
                                engine=inst.engine,
                                ins=[],
                                outs=[],
                                sync_info=bass_rust.SyncInfo(
                                    on_wait=[w], on_update=[]
                                ),
                            )
                        )
                    inst.sync_info = bass_rust.SyncInfo(
                        on_wait=waits[:cap], on_update=list(si.on_update)
                    )
                new_insts.append(inst)
            bb.instructions = new_insts
    return n


def _assert_contiguous_acc_groups(nc):
    """The final (post-Tile-scheduling) tensor-engine stream must not carry
    a foreign matmul inside an open PSUM accumulation group: that pattern
    faults this hardware.  The schedule is fixed at compile time, so a
    static scan is a complete check."""
    import concourse.mybir as mybir

    for fn in nc.m.functions:
        for bb in fn.blocks:
            open_out = None
            for inst in bb.instructions:
                if not isinstance(inst, mybir.InstMatmult):
                    continue
                out_name = inst.outs[0].memref
                if open_out is not None and out_name != open_out:
                    raise AssertionError(
                        f"foreign matmul {inst.name} -> {out_name} inside "
                        f"open accumulation group on {open_out}"
                    )
                if inst.start_tensor_calc and not inst.stop_tensor_calc:
                    open_out = out_name
                elif inst.stop_tensor_calc:
                    open_out = None


def _build(split_waits=True, mm1_accumulate=True, defer_mm2=True, use_gate2=False):
    import concourse.bass as bass
    import concourse.mybir as mybir
    from concourse.masks import make_identity
    from concourse.tile import TileContext

    _apply_tile_drain_patch()
    f32 = mybir.dt.float32
    bf16 = mybir.dt.bfloat16
    i32 = mybir.dt.int32

    nc = bass.Bass()
    z = nc.declare_dram_parameter("z", [_K, _M, _D], f32, isOutput=False)
    # A_all [64, 1024, 16] viewed as rows (e, dc) of [128, 16] blocks
    a_tab = nc.declare_dram_parameter("a_tab", [64 * 8, 128 * _R], f32, isOutput=False)
    # B_all [64, 16, 1024] viewed as rows (e, r) of [1024] d-vectors
    b_tab = nc.declare_dram_parameter("b_tab", [64 * _R, _D], f32, isOutput=False)
    idxa = nc.declare_dram_parameter("idxa", [64, 1], i32, isOutput=False)
    idxb = nc.declare_dram_parameter("idxb", [128, 1], i32, isOutput=False)
    out = nc.declare_dram_parameter("out", [_K, _M, _D], f32, isOutput=True)

    with TileContext(nc) as tc:
        with (
            tc.tile_pool(name="const", bufs=1) as cpool,
            tc.tile_pool(name="zin", bufs=4) as zpool,
            tc.tile_pool(name="io", bufs=3) as iopool,
            tc.tile_pool(name="acc", bufs=2) as apool,
            tc.tile_pool(name="psd", bufs=2, space="PSUM") as psd,
            tc.tile_pool(name="psu", bufs=2, space="PSUM") as psu,
            tc.tile_pool(name="pst", bufs=2, space="PSUM") as pst,
        ):
            def load_zb(k, split=False):
                # SWDGE cast-DMA f32 HBM -> bf16 SBUF; partition p reads
                # rows 4p..4p+3 as ONE contiguous 16 KiB run (m = 4*mp+ms),
                # so descriptor count stays at 128 (the SWDGE ring and
                # descriptor-generation cost scale with run count).
                # split=True (first loads only) lands rows 4p..4p+1 early
                # so the (dj, ms<2) transposes start ~3us sooner.
                zb = zpool.tile([128, 4096], bf16, tag="zb")
                v = z[k].rearrange("(p q) d -> p (q d)", p=128)
                if split:
                    nc.gpsimd.dma_start(out=zb[:, 0:2048], in_=v[:, 0:2048])
                    nc.gpsimd.dma_start(out=zb[:, 2048:4096], in_=v[:, 2048:4096])
                else:
                    nc.gpsimd.dma_start(out=zb[:], in_=v[:])
                return zb

            ident = cpool.tile([128, 128], bf16)
            make_identity(nc, ident[:])

            # ---- one-time expert gather + layout prep ----
            ia = cpool.tile([64, 1], i32)
            nc.sync.dma_start(out=ia[:], in_=idxa[:])

            # z loads lead the SWDGE queue: transposes for k=0,1 are the
            # first consumers, and the early iterations otherwise run the
            # load train just-in-time.  The A gather + prep still complete
            # before mm1(0) needs a_tb.
            zb_pre = [load_zb(0, split=True), load_zb(1, split=True)]

            # gather A rows (k,dc) -> [64, 2048]; row content is [128p, 16r]
            a_rows = cpool.tile([64, 2048], f32)
            nc.gpsimd.indirect_dma_start(
                out=a_rows[:],
                out_offset=None,
                in_=a_tab[:],
                in_offset=bass.IndirectOffsetOnAxis(ap=ia[:, :1], axis=0),
            )
            # identity for PE transposes (f32 copy for the A-table prep)
            identf = cpool.tile([128, 128], f32)
            make_identity(nc, identf[:])
            # redistribute d across partitions with 16 strided PE
            # transposes (one per rank index r): [64(k,dc), 128(d)] -> psum
            # [128(d), (r, k, dc)], then one free-dim-permuted DVE copy with
            # the LoRA scale and bf16 cast folded in.
            a_rows_v = a_rows[:].rearrange("j (p r) -> j r p", r=_R)
            pa = psu.tile([128, 1024], f32, tag="up")
            for r in range(_R):
                nc.tensor.transpose(
                    out=pa[:, r * 64 : (r + 1) * 64],
                    in_=a_rows_v[:, r, :],
                    identity=identf[:64, :64],
                )
            a_tb = cpool.tile([128, 8 * 8 * _R], bf16)
            nc.vector.tensor_scalar_mul(
                a_tb[:].rearrange("p (k dc r) -> p r (k dc)", k=8, r=_R),
                pa[:].rearrange("p (r j) -> p r j", r=_R),
                _SCALE,
            )

            zb_pre.append(load_zb(2))

            # gather B rows (e,r) -> per-k [16r, 1024d] bf16 tiles, casting
            # in the DMA.  The offset AP and the matmul operands must sit
            # at SBUF base partition 0 (nonzero-base offset APs fault the
            # hardware), hence per-k index tiles.  Gathers are spread one
            # per iteration: packing them upfront clogs the SWDGE ring and
            # delays the z-load completion semaphores by several us.
            b_kt = [None] * _K

            def gather_b(k):
                ibk = cpool.tile([16, 1], i32, tag=f"ibk{k}")
                nc.sync.dma_start(out=ibk[:], in_=idxb[k * 16 : (k + 1) * 16, :])
                btb = cpool.tile([16, _D], bf16, tag=f"bb{k}")
                nc.gpsimd.indirect_dma_start(
                    out=btb[:],
                    out_offset=None,
                    in_=b_tab[:],
                    in_offset=bass.IndirectOffsetOnAxis(ap=ibk[:, :1], axis=0),
                )
                b_kt[k] = btb


            def emit_mm2(k, db):
                # mm2 + copy out, per-chunk stores.  psu partition mp maps
                # to output row m = 4*mp + ms for chunk ms.
                ov = None
                for ms in range(4):
                    pu = psu.tile([128, 1024], f32, tag="up")
                    for dc2 in range(2):
                        nc.tensor.matmul(
                            out=pu[:, dc2 * 512 : (dc2 + 1) * 512],
                            lhsT=db[:, ms * 128 : (ms + 1) * 128],
                            rhs=b_kt[k][:, dc2 * 512 : (dc2 + 1) * 512],
                            start=True,
                            stop=True,
                        )
                    ov = iopool.tile([128, 1024], f32, tag="ov")
                    nc.vector.tensor_copy(out=ov[:, 0:512], in_=pu[:, 0:512])
                    nc.scalar.copy(out=ov[:, 512:1024], in_=pu[:, 512:1024])
                    nc.sync.dma_start(
                        out=out[k].rearrange("(p q) d -> p q d", p=128)[:, ms, :],
                        in_=ov[:],
                    )
                return ov

            # ---- main loop over the K selected experts ----
            db_prev = None
            for k in range(_K):
                zb = zb_pre[k]
                if k + 3 < _K:
                    zb_pre.append(load_zb(k + 3))
                if k == 0:
                    gather_b(0)
                if k + 1 < _K:
                    gather_b(k + 1)

                # z^T chunks via PE transpose (bf16, 1 cycle/row):
                #   zt[p, dc*512 + ms*128 + mp] = zb[mp, ms*1024 + dc*128 + p]
                zt = iopool.tile([128, 4096], bf16, tag="zt")
                for dh in range(4):
                    pt = pst.tile([128, 1024], bf16, tag="zt_ps")
                    for dj in range(2):
                        dc = dh * 2 + dj
                        for ms in range(4):
                            nc.tensor.transpose(
                                out=pt[:, dj * 512 + ms * 128 : dj * 512 + (ms + 1) * 128],
                                in_=zb[:, ms * 1024 + dc * 128 : ms * 1024 + (dc + 1) * 128],
                                identity=ident[:],
                            )
                    # drain each psum tile with both engines (halves) so
                    # the tile frees 2x sooner and neither engine hogs
                    nc.vector.tensor_copy(
                        out=zt[:, dh * 1024 : dh * 1024 + 512], in_=pt[:, 0:512]
                    )
                    nc.scalar.copy(
                        out=zt[:, dh * 1024 + 512 : (dh + 1) * 1024], in_=pt[:, 512:1024]
                    )

                # With use_gate2, mm2(k-1) is emitted BEFORE mm1(k)'s
                # accumulation group (scheduler drains it first) and the
                # group is additionally gated on its final PSUM drain.
                # Without it, mm2(k-1) is emitted after the group (lower
                # priority) -- usually schedules cleanly and faster; the
                # static group check decides at build time which is safe.
                ov_last = None
                if use_gate2 and defer_mm2 and db_prev is not None:
                    ov_last = emit_mm2(k - 1, db_prev)

                # mm1: down^T [16, 512] accumulated in PSUM over 8 d-chunks
                if mm1_accumulate:
                    pd = psd.tile([16, 512], f32, tag="down")
                    if ov_last is not None:
                        # gate 2: block the group until mm2(k-1)'s final
                        # PSUM drain lands -- its matmuls then cannot slip
                        # into the open group (hardware fault)
                        nc.tensor.transpose(
                            out=pd[0:8, 0:128],
                            in_=ov_last[:, 0:8],
                            identity=identf[:],
                        )
                    # gate: throwaway PE transpose reading one column of
                    # each zt chunk (see module docstring)
                    nc.tensor.transpose(
                        out=pd[0:8, 0:64].bitcast(bf16),
                        in_=zt[:].rearrange("p (c q) -> p c q", c=8)[:, :, 0],
                        identity=ident[:],
                    )
                    for dc in range(8):
                        nc.tensor.matmul(
                            out=pd[:],
                            lhsT=a_tb[:, (k * 8 + dc) * _R : (k * 8 + dc + 1) * _R],
                            rhs=zt[:, dc * 512 : (dc + 1) * 512],
                            start=(dc == 0),
                            stop=(dc == 7),
                        )
                    db = apool.tile([16, 512], bf16, tag="db")
                    nc.scalar.copy(out=db[:], in_=pd[:])
                else:
                    # fallback: singleton matmuls + add tree off the PE
                    t4 = []
                    for dc in range(8):
                        pdx = psd.tile([16, 512], f32, tag="down")
                        nc.tensor.matmul(
                            out=pdx[:],
                            lhsT=a_tb[:, (k * 8 + dc) * _R : (k * 8 + dc + 1) * _R],
                            rhs=zt[:, dc * 512 : (dc + 1) * 512],
                            start=True,
                            stop=True,
                        )
                        if dc % 2 == 0:
                            t = apool.tile([16, 512], f32, tag=f"t{dc // 2}")
                            nc.scalar.copy(out=t[:], in_=pdx[:])
                            t4.append(t)
                        else:
                            t = t4[dc // 2]
                            nc.vector.tensor_add(out=t[:], in0=t[:], in1=pdx[:])
                    u0 = apool.tile([16, 512], f32, tag="u0")
                    nc.gpsimd.tensor_add(out=u0[:], in0=t4[0][:], in1=t4[1][:])
                    u1 = apool.tile([16, 512], f32, tag="u1")
                    nc.gpsimd.tensor_add(out=u1[:], in0=t4[2][:], in1=t4[3][:])
                    db = apool.tile([16, 512], bf16, tag="db")
                    nc.vector.tensor_add(out=db[:], in0=u0[:], in1=u1[:])

                if defer_mm2:
                    if not use_gate2 and db_prev is not None:
                        emit_mm2(k - 1, db_prev)
                    db_prev = db
                else:
                    emit_mm2(k, db)

            if defer_mm2 and db_prev is not None:
                emit_mm2(_K - 1, db_prev)

    if split_waits:
        _split_excess_waits(nc)
    import os as _os
    if _os.environ.get("ALLOW_INTERLEAVE", "0") != "1":
        _assert_contiguous_acc_groups(nc)
    return nc


def kernel(z, A_all, B_all, expert_indices, _trace=False):
    import os

    from concourse.bass_utils import run_bass_kernel_spmd

    z = np.ascontiguousarray(np.asarray(z, dtype=np.float32))
    A_all = np.ascontiguousarray(np.asarray(A_all, dtype=np.float32))
    B_all = np.ascontiguousarray(np.asarray(B_all, dtype=np.float32))
    idx = np.asarray(expert_indices).astype(np.int64)
    assert z.shape == (_B, _K, _M, _D)

    key = (
        os.environ.get("MM1_ACC", "1") == "1",
        os.environ.get("DEFER_MM2", "1") == "1",
        os.environ.get("USE_GATE2", "auto"),
    )
    if key not in _cache:
        if key[2] == "auto":
            # The PSUM accumulation group must schedule with no foreign
            # matmul inside it (hardware fault).  Prefer the faster
            # no-gate2 emission; fall back to the strictly-gated one if
            # the static schedule check trips.
            try:
                _cache[key] = _build(
                    mm1_accumulate=key[0], defer_mm2=key[1], use_gate2=False
                )
            except AssertionError:
                _cache[key] = _build(
                    mm1_accumulate=key[0], defer_mm2=key[1], use_gate2=True
                )
        else:
            _cache[key] = _build(
                mm1_accumulate=key[0], defer_mm2=key[1], use_gate2=key[2] == "1"
            )
    nc = _cache[key]

    a_tab = A_all.reshape(64 * 8, 128 * _R)
    b_tab = B_all.reshape(64 * _R, _D)
    idxa = (idx[:, None] * 8 + np.arange(8)[None, :]).reshape(64, 1).astype(np.int32)
    idxb = (idx[:, None] * 16 + np.arange(16)[None, :]).reshape(128, 1).astype(np.int32)

    in_maps = [
        {"z": z[c], "a_tab": a_tab, "b_tab": b_tab, "idxa": idxa, "idxb": idxb}
        for c in range(_NCORES)
    ]
    res = run_bass_kernel_spmd(nc, in_maps, list(range(_NCORES)), trace=_trace)
    globals()["last_exec_time_ns"] = res.exec_time_ns
    return np.stack([res.results[c]["out"] for c in range(_NCORES)], axis=0)


# revision 67
# speedup vs baseline: 1.0694x; 1.0694x over previous
"""Trainium2 Bass kernel for ExpertMLPLoRA (moe_routing).

Reference computation (per batch b, selected expert k):
    A = A_all[expert_indices]            # [K, D, R]
    Bm = B_all[expert_indices]           # [K, R, D]
    down = einsum('bkmd,kdr->bkmr', z, A)
    up   = einsum('bkmr,krd->bkmd', down, Bm)
    out  = up * (alpha/rank)

Sharding: data-parallel over batch B=8 -> one batch per NeuronCore.
Each core receives its z[b] slice plus the full (replicated) LoRA
tables and gathers the K=8 selected experts on-device via indirect
DMA.  Host only pre-expands the 8 expert indices into row indices
(pure address arithmetic).

Device pipeline per (b, k):
  1. SWDGE cast-DMA z[b,k] [512, 1024] f32 HBM -> bf16 SBUF with one
     fully-contiguous 16 KiB read run per partition (partition p holds
     rows 4p..4p+3, i.e. m = 4*mp + ms) -- minimal descriptor count
  2. 32x PE transpose (bf16) -> z^T chunks in PSUM, copied to SBUF on
     DVE/ACT; the m axis stays in the permuted (ms, mp) order
  3. mm1: one PSUM accumulation group of 8 matmuls
     A_chunk[128d,16r].T @ z^T[128d,512m] += down^T.  A throwaway PE
     transpose reading one column of every z^T chunk gates the group:
     every member is ready the moment the group opens, so the tile
     scheduler cannot interleave a foreign matmul into the open group
     (that faults this hardware; _assert_contiguous_acc_groups checks
     the final schedule statically).
  4. one PSUM->SBUF bf16 copy of down^T (LoRA scale folded into the
     gathered A table)
  5. mm2 per m-chunk ms: up[128mp, 1024d] = down^T[16,128].T @
     B_k[16,1024]; PSUM->SBUF copy, then a 512 KiB store with the
     permuted row mapping out[k, 4*mp+ms, :]
"""

import numpy as np

_B, _K, _M, _D, _R = 8, 8, 512, 1024, 16
_SCALE = 1.0 / _R
_NCORES = 8

_cache = {}


def _apply_tile_drain_patch():
    """This walrus build caps sync waits at 1 per instruction (2 for
    EventSemaphore).  Tile's kernel-tail drain piles every final sem wait
    onto one Drain -> NCC_INLA001 'Too many sync wait commands'.  Re-emit
    the extras as standalone per-sem waits before the drain."""
    import concourse.tile as tile_mod
    from concourse.tile import TileContext

    if getattr(TileContext, "_drain_patch_applied", False):
        return
    try:
        from concourse.tile import ScopedClock
    except ImportError:
        from bass_rust import ScopedClock

    def _patched(self, tick_clock, wait_clock):
        nc = self.nc
        probe = nc.sync.drain()
        wait_clock.add_sem_waits(
            probe.ins, ScopedClock({None: tick_clock.global_clock})
        )
        waits = list(probe.ins.sync_info.on_wait)
        if len(waits) > 1:
            assert self.sems is not None
            by_name = {s.name: s for s in self.sems.allocated().values()}
            for w in waits[1:]:
                sem = by_name.get(w.ant_name)
                assert sem is not None, f"semaphore {w.ant_name} not found"
                nc.sync.wait_ge(sem, w.wait_value)
            probe.ins.sync_info.on_wait = waits[:1]
            nc.sync.drain()
        nc.all_engine_barrier()
        assert self.sems is not None
        popped = nc._tile_sem_poison_stack.pop()
        assert popped is self._sem_poison
        # End-of-kernel sem/DMA-queue scrubbing (gpsimd dma_reset +
        # sem_clear + a second all-engine barrier) is only needed if the
        # NEFF re-executes; this kernel is single-shot, so do only the
        # Python-side bookkeeping of clear_and_free_semaphores and skip
        # the ~3-5us of teardown instructions.
        sems = list(self.sems.allocated().values())
        sem_nums = [s.num for s in sems]
        if sem_nums:
            nc._state.prepend_free_semaphores(sem_nums)
            for poison_set in nc._tile_sem_poison_stack:
                poison_set.update(sem_nums)

    TileContext._drain_and_barrier = _patched
    TileContext._drain_patch_applied = True


def _split_excess_waits(nc):
    """This walrus build rejects instructions carrying more than 1-2 sync
    waits ('Too many sync wait commands'), but Tile's sem-assignment packs
    up to ~9 waits onto one instruction.  Hoist the excess onto standalone
    EventSemaphore carriers placed immediately before the instruction on
    the same engine (engines execute in order, so blocking semantics are
    identical)."""
    import bass_rust
    import concourse.mybir as mybir

    n = 0
    for fn in nc.m.functions:
        for bb in fn.blocks:
            new_insts = []
            for inst in bb.instructions:
                si = inst.sync_info
                waits = list(si.on_wait) if si is not None else []
                cap = 2 if isinstance(inst, mybir.InstEventSemaphore) else 1
                if len(waits) > cap:
                    for w in waits[cap:]:
                        n += 1
                        new_insts.append(
                            mybir.InstEventSemaphore(
                                name=f"wsplit-{n}-{inst.name}",
                                engine=inst.engine,
                                ins=[],
                                outs=[],
                                sync_info=bass_rust.SyncInfo(
                                    on_wait=[w], on_update=[]
                                ),
                            )
                        )
                    inst.sync_info = bass_rust.SyncInfo(
                        on_wait=waits[:cap], on_update=list(si.on_update)
                    )
                new_insts.append(inst)
            bb.instructions = new_insts
    return n


def _assert_contiguous_acc_groups(nc):
    """The final (post-Tile-scheduling) tensor-engine stream must not carry
    a foreign matmul inside an open PSUM accumulation group: that pattern
    faults this hardware.  The schedule is fixed at compile time, so a
    static scan is a complete check."""
    import concourse.mybir as mybir

    for fn in nc.m.functions:
        for bb in fn.blocks:
            open_out = None
            for inst in bb.instructions:
                if not isinstance(inst, mybir.InstMatmult):
                    continue
                out_name = inst.outs[0].memref
                if open_out is not None and out_name != open_out:
                    raise AssertionError(
                        f"foreign matmul {inst.name} -> {out_name} inside "
                        f"open accumulation group on {open_out}"
                    )
                if inst.start_tensor_calc and not inst.stop_tensor_calc:
                    open_out = out_name
                elif inst.stop_tensor_calc:
                    open_out = None


def _build(split_waits=True, mm1_accumulate=True, defer_mm2=True, use_gate2=False):
    import concourse.bass as bass
    import concourse.mybir as mybir
    from concourse.masks import make_identity
    from concourse.tile import TileContext

    _apply_tile_drain_patch()
    f32 = mybir.dt.float32
    bf16 = mybir.dt.bfloat16
    i32 = mybir.dt.int32

    nc = bass.Bass()
    z = nc.declare_dram_parameter("z", [_K, _M, _D], f32, isOutput=False)
    # A_all [64, 1024, 16] viewed as rows (e, dc) of [128, 16] blocks
    a_tab = nc.declare_dram_parameter("a_tab", [64 * 8, 128 * _R], f32, isOutput=False)
    # B_all [64, 16, 1024] viewed as rows (e, r) of [1024] d-vectors
    b_tab = nc.declare_dram_parameter("b_tab", [64 * _R, _D], f32, isOutput=False)
    idxa = nc.declare_dram_parameter("idxa", [64, 1], i32, isOutput=False)
    idxb = nc.declare_dram_parameter("idxb", [128, 1], i32, isOutput=False)
    out = nc.declare_dram_parameter("out", [_K, _M, _D], f32, isOutput=True)

    with TileContext(nc) as tc:
        with (
            tc.tile_pool(name="const", bufs=1) as cpool,
            tc.tile_pool(name="zin", bufs=4) as zpool,
            tc.tile_pool(name="io", bufs=3) as iopool,
            tc.tile_pool(name="acc", bufs=2) as apool,
            tc.tile_pool(name="psd", bufs=2, space="PSUM") as psd,
            tc.tile_pool(name="psu", bufs=2, space="PSUM") as psu,
            tc.tile_pool(name="pst", bufs=2, space="PSUM") as pst,
        ):
            def load_zb(k, split=False):
                # SWDGE cast-DMA f32 HBM -> bf16 SBUF; partition p reads
                # rows 4p..4p+3 as ONE contiguous 16 KiB run (m = 4*mp+ms),
                # so descriptor count stays at 128 (the SWDGE ring and
                # descriptor-generation cost scale with run count).
                # split=True (first loads only) lands rows 4p..4p+1 early
                # so the (dj, ms<2) transposes start ~3us sooner.
                zb = zpool.tile([128, 4096], bf16, tag="zb")
                v = z[k].rearrange("(p q) d -> p (q d)", p=128)
                if split:
                    nc.gpsimd.dma_start(out=zb[:, 0:2048], in_=v[:, 0:2048])
                    nc.gpsimd.dma_start(out=zb[:, 2048:4096], in_=v[:, 2048:4096])
                else:
                    nc.gpsimd.dma_start(out=zb[:], in_=v[:])
                return zb

            ident = cpool.tile([128, 128], bf16)
            make_identity(nc, ident[:])

            # ---- one-time expert gather + layout prep ----
            ia = cpool.tile([64, 1], i32)
            nc.sync.dma_start(out=ia[:], in_=idxa[:])

            # z loads lead the SWDGE queue: transposes for k=0,1 are the
            # first consumers, and the early iterations otherwise run the
            # load train just-in-time.  The A gather + prep still complete
            # before mm1(0) needs a_tb.
            zb_pre = [load_zb(0, split=True), load_zb(1, split=True)]

            # gather A rows (k,dc) -> [64, 2048]; row content is [128p, 16r]
            a_rows = cpool.tile([64, 2048], f32)
            nc.gpsimd.indirect_dma_start(
                out=a_rows[:],
                out_offset=None,
                in_=a_tab[:],
                in_offset=bass.IndirectOffsetOnAxis(ap=ia[:, :1], axis=0),
            )
            # identity for PE transposes (f32 copy for the A-table prep)
            identf = cpool.tile([128, 128], f32)
            make_identity(nc, identf[:])
            # redistribute d across partitions with 16 strided PE
            # transposes (one per rank index r): [64(k,dc), 128(d)] -> psum
            # [128(d), (r, k, dc)], then one free-dim-permuted DVE copy with
            # the LoRA scale and bf16 cast folded in.
            a_rows_v = a_rows[:].rearrange("j (p r) -> j r p", r=_R)
            pa = psu.tile([128, 1024], f32, tag="up")
            for r in range(_R):
                nc.tensor.transpose(
                    out=pa[:, r * 64 : (r + 1) * 64],
                    in_=a_rows_v[:, r, :],
                    identity=identf[:64, :64],
                )
            a_tb = cpool.tile([128, 8 * 8 * _R], bf16)
            nc.vector.tensor_scalar_mul(
                a_tb[:].rearrange("p (k dc r) -> p r (k dc)", k=8, r=_R),
                pa[:].rearrange("p (r j) -> p r j", r=_R),
                _SCALE,
            )

            zb_pre.append(load_zb(2))

            # gather B rows (e,r) -> per-k [16r, 1024d] bf16 tiles, casting
            # in the DMA.  The offset AP and the matmul operands must sit
            # at SBUF base partition 0 (nonzero-base offset APs fault the
            # hardware), hence per-k index tiles.  Gathers are spread one
            # per iteration: packing them upfront clogs the SWDGE ring and
            # delays the z-load completion semaphores by several us.
            b_kt = [None] * _K

            def gather_b(k):
                ibk = cpool.tile([16, 1], i32, tag=f"ibk{k}")
                nc.sync.dma_start(out=ibk[:], in_=idxb[k * 16 : (k + 1) * 16, :])
                btb = cpool.tile([16, _D], bf16, tag=f"bb{k}")
                nc.gpsimd.indirect_dma_start(
                    out=btb[:],
                    out_offset=None,
                    in_=b_tab[:],
                    in_offset=bass.IndirectOffsetOnAxis(ap=ibk[:, :1], axis=0),
                )
                b_kt[k] = btb


            def emit_mm2(k, db):
                # mm2 + copy out, per-chunk stores.  psu partition mp maps
                # to output row m = 4*mp + ms for chunk ms.
                ov = None
                for ms in range(4):
                    pu = psu.tile([128, 1024], f32, tag="up")
                    for dc2 in range(2):
                        nc.tensor.matmul(
                            out=pu[:, dc2 * 512 : (dc2 + 1) * 512],
                            lhsT=db[:, ms * 128 : (ms + 1) * 128],
                            rhs=b_kt[k][:, dc2 * 512 : (dc2 + 1) * 512],
                            start=True,
                            stop=True,
                        )
                    ov = iopool.tile([128, 1024], f32, tag="ov")
                    nc.vector.tensor_copy(out=ov[:, 0:512], in_=pu[:, 0:512])
                    nc.scalar.copy(out=ov[:, 512:1024], in_=pu[:, 512:1024])
                    nc.sync.dma_start(
                        out=out[k].rearrange("(p q) d -> p q d", p=128)[:, ms, :],
                        in_=ov[:],
                    )
                return ov

            # ---- main loop over the K selected experts ----
            db_prev = None
            for k in range(_K):
                zb = zb_pre[k]
                if k + 3 < _K:
                    zb_pre.append(load_zb(k + 3))
                gather_b(k)

                # z^T chunks via PE transpose (bf16, 1 cycle/row):
                #   zt[p, dc*512 + ms*128 + mp] = zb[mp, ms*1024 + dc*128 + p]
                zt = iopool.tile([128, 4096], bf16, tag="zt")
                for dh in range(4):
                    pt = pst.tile([128, 1024], bf16, tag="zt_ps")
                    for dj in range(2):
                        dc = dh * 2 + dj
                        for ms in range(4):
                            nc.tensor.transpose(
                                out=pt[:, dj * 512 + ms * 128 : dj * 512 + (ms + 1) * 128],
                                in_=zb[:, ms * 1024 + dc * 128 : ms * 1024 + (dc + 1) * 128],
                                identity=ident[:],
                            )
                    # drain each psum tile with both engines (halves) so
                    # the tile frees 2x sooner and neither engine hogs
                    nc.vector.tensor_copy(
                        out=zt[:, dh * 1024 : dh * 1024 + 512], in_=pt[:, 0:512]
                    )
                    nc.scalar.copy(
                        out=zt[:, dh * 1024 + 512 : (dh + 1) * 1024], in_=pt[:, 512:1024]
                    )

                # With use_gate2, mm2(k-1) is emitted BEFORE mm1(k)'s
                # accumulation group (scheduler drains it first) and the
                # group is additionally gated on its final PSUM drain.
                # Without it, mm2(k-1) is emitted after the group (lower
                # priority) -- usually schedules cleanly and faster; the
                # static group check decides at build time which is safe.
                ov_last = None
                if use_gate2 and defer_mm2 and db_prev is not None:
                    ov_last = emit_mm2(k - 1, db_prev)

                # mm1: down^T [16, 512] accumulated in PSUM over 8 d-chunks
                if mm1_accumulate:
                    pd = psd.tile([16, 512], f32, tag="down")
                    if ov_last is not None:
                        # gate 2: block the group until mm2(k-1)'s final
                        # PSUM drain lands -- its matmuls then cannot slip
                        # into the open group (hardware fault)
                        nc.tensor.transpose(
                            out=pd[0:8, 0:128],
                            in_=ov_last[:, 0:8],
                            identity=identf[:],
                        )
                    # gate: throwaway PE transpose reading one column of
                    # each zt chunk (see module docstring)
                    nc.tensor.transpose(
                        out=pd[0:8, 0:64].bitcast(bf16),
                        in_=zt[:].rearrange("p (c q) -> p c q", c=8)[:, :, 0],
                        identity=ident[:],
                    )
                    for dc in range(8):
                        nc.tensor.matmul(
                            out=pd[:],
                            lhsT=a_tb[:, (k * 8 + dc) * _R : (k * 8 + dc + 1) * _R],
                            rhs=zt[:, dc * 512 : (dc + 1) * 512],
                            start=(dc == 0),
                            stop=(dc == 7),
                        )
                    db = apool.tile([16, 512], bf16, tag="db")
                    nc.scalar.copy(out=db[:], in_=pd[:])
                else:
                    # fallback: singleton matmuls + add tree off the PE
                    t4 = []
                    for dc in range(8):
                        pdx = psd.tile([16, 512], f32, tag="down")
                        nc.tensor.matmul(
                            out=pdx[:],
                            lhsT=a_tb[:, (k * 8 + dc) * _R : (k * 8 + dc + 1) * _R],
                            rhs=zt[:, dc * 512 : (dc + 1) * 512],
                            start=True,
                            stop=True,
                        )
                        if dc % 2 == 0:
                            t = apool.tile([16, 512], f32, tag=f"t{dc // 2}")
                            nc.scalar.copy(out=t[:], in_=pdx[:])
                            t4.append(t)
                        else:
                            t = t4[dc // 2]
                            nc.vector.tensor_add(out=t[:], in0=t[:], in1=pdx[:])
                    u0 = apool.tile([16, 512], f32, tag="u0")
                    nc.gpsimd.tensor_add(out=u0[:], in0=t4[0][:], in1=t4[1][:])
                    u1 = apool.tile([16, 512], f32, tag="u1")
                    nc.gpsimd.tensor_add(out=u1[:], in0=t4[2][:], in1=t4[3][:])
                    db = apool.tile([16, 512], bf16, tag="db")
                    nc.vector.tensor_add(out=db[:], in0=u0[:], in1=u1[:])

                if defer_mm2:
                    if not use_gate2 and db_prev is not None:
                        emit_mm2(k - 1, db_prev)
                    db_prev = db
                else:
                    emit_mm2(k, db)

            if defer_mm2 and db_prev is not None:
                emit_mm2(_K - 1, db_prev)

    if split_waits:
        _split_excess_waits(nc)
    import os as _os
    if _os.environ.get("ALLOW_INTERLEAVE", "0") != "1":
        _assert_contiguous_acc_groups(nc)
    return nc


def kernel(z, A_all, B_all, expert_indices, _trace=False):
    import os

    from concourse.bass_utils import run_bass_kernel_spmd

    z = np.ascontiguousarray(np.asarray(z, dtype=np.float32))
    A_all = np.ascontiguousarray(np.asarray(A_all, dtype=np.float32))
    B_all = np.ascontiguousarray(np.asarray(B_all, dtype=np.float32))
    idx = np.asarray(expert_indices).astype(np.int64)
    assert z.shape == (_B, _K, _M, _D)

    key = (
        os.environ.get("MM1_ACC", "1") == "1",
        os.environ.get("DEFER_MM2", "1") == "1",
        os.environ.get("USE_GATE2", "auto"),
    )
    if key not in _cache:
        if key[2] == "auto":
            # The PSUM accumulation group must schedule with no foreign
            # matmul inside it (hardware fault).  Prefer the faster
            # no-gate2 emission; fall back to the strictly-gated one if
            # the static schedule check trips.
            try:
                _cache[key] = _build(
                    mm1_accumulate=key[0], defer_mm2=key[1], use_gate2=False
                )
            except AssertionError:
                _cache[key] = _build(
                    mm1_accumulate=key[0], defer_mm2=key[1], use_gate2=True
                )
        else:
            _cache[key] = _build(
                mm1_accumulate=key[0], defer_mm2=key[1], use_gate2=key[2] == "1"
            )
    nc = _cache[key]

    a_tab = A_all.reshape(64 * 8, 128 * _R)
    b_tab = B_all.reshape(64 * _R, _D)
    idxa = (idx[:, None] * 8 + np.arange(8)[None, :]).reshape(64, 1).astype(np.int32)
    idxb = (idx[:, None] * 16 + np.arange(16)[None, :]).reshape(128, 1).astype(np.int32)

    in_maps = [
        {"z": z[c], "a_tab": a_tab, "b_tab": b_tab, "idxa": idxa, "idxb": idxb}
        for c in range(_NCORES)
    ]
    res = run_bass_kernel_spmd(nc, in_maps, list(range(_NCORES)), trace=_trace)
    globals()["last_exec_time_ns"] = res.exec_time_ns
    return np.stack([res.results[c]["out"] for c in range(_NCORES)], axis=0)
